# revision 1
# baseline (speedup 1.0000x reference)
"""Trainium2 kernel for per-subject linear heads (moe_routing).

Computes out[i] = x[i] @ W[subject_ids[i]] + b[subject_ids[i]] for
B=256, D=2048, S=8 subjects, OUT=1000.

Sharding: expert-parallel — core s owns subject s. Each core reads only
its own (2048, 1000) fp32 weight slice (8.19 MB) from HBM, so the total
weight traffic across the chip is W read exactly once (vs 8x for
batch-data-parallel with a replicated table). Samples are grouped by
subject on the host, padded to a fixed capacity C, and fed to an SPMD
Bass/Tile kernel; outputs are scattered back to the original order.

Kernel-side notes:
- The bias is folded into the matmul accumulation as a rank-1 update
  (ones row carried as an extra k-slot of x, times the [1, OUT] bias).
- This walrus build rejects any instruction with more than one sync
  wait, so the kernel is structured so no instruction ever needs two:
  a scratch-fed warm-up matmul absorbs the x-DMA wait, exactly 8 DMAs
  map 1:1 onto the 8 HWDGE completion-sem lanes, and the custom
  TileContext tail emits one drain per semaphore.
- W is pre-permuted on the host so each chunk DMA reads one contiguous
  16 KB run per partition (128 fat descriptors instead of 512).
- A chain of throwaway matmuls on a memset scratch tile keeps the PE
  busy from kernel start so the HAM clock-gate reaches 2.4 GHz before
  the real matmul stream begins.
"""

import numpy as np

import concourse.bass as bass
import concourse.mybir as mybir
import concourse.tile as tile
from concourse.bass_utils import run_bass_kernel_spmd
from concourse.vector_clock import ScopedClock, VectorClock

B = 256
D = 2048
S = 8
OUT = 1000
P = 128
KO = D // P          # 16 k-tiles of 128
NT = 500             # psum n-tile (<= 512 fp32 / bank), 2 tiles cover OUT
CH = 2               # k-tiles per W DMA chunk (2 * 128 * 1000 * 4B = 1 MB)
N_CHUNKS = KO // CH

USE_FP32R = False    # fp32r: ~4x faster PE, ~1e-4 rel err (HW truncates)
SPINS_PRE = 16       # PE warm-up matmuls before the real stream
                     # (each fp32 spin = 4 PE instructions; keep the
                     # PE code block within one 256-instruction IRAM
                     # block to avoid a mid-stream ifetch stall)
SPIN_N = 128         # spin matmul free dim (short, so cut-over to real work is fast)
SPINS_GAP = 0        # filler matmuls between chunk groups

TRACE = False        # set by test harness to collect an NTFF profile
LAST_RESULTS = None  # BassKernelResults of the most recent run

_nc_cache = {}


class _FastExitTileContext(tile.TileContext):
    """TileContext with a single-wait-per-instruction, barrier-free exit.

    This walrus build rejects instructions with >1 sync wait, and the
    stock exit (one Drain waiting on every semaphore + two all-engine
    EVSEM-butterfly barriers) both violates that and costs ~8 us. Here
    SP emits one drain per logical processor (each <=1 wait), then
    hands off to GpSimd via a fresh semaphore; GpSimd resets the DMA
    queues and clears all Tile semaphores (required so a re-execution
    of the NEFF starts from zeroed sems). By the time SP's drains have
    observed every semaphore at its final value, every engine has
    retired its last instruction, so the butterfly barriers are
    unnecessary.
    """

    def _drain_and_barrier(self, tick_clock, wait_clock):
        nc = self.nc
        gc = tick_clock.global_clock
        n = len(gc)
        last = None
        for i in range(n):
            if gc[i] <= 0:
                continue
            vec = [0] * n
            vec[i] = gc[i]
            d = nc.sync.drain()
            wait_clock.add_sem_waits(d.ins, ScopedClock({None: VectorClock(vec)}))
            last = d

        assert self.sems is not None
        popped = nc._tile_sem_poison_stack.pop()
        assert popped is self._sem_poison
        sems = list(self.sems.allocated().values())
        if last is not None:
            handoff = nc.alloc_semaphore(name="exit_handoff")
            last.then_inc(handoff, 1)
            nc.gpsimd.wait_ge(handoff, 1)
            nc.clear_and_free_semaphores(sems)
            nc.gpsimd.sem_clear(handoff)
            nc.release_semaphore(handoff)
        else:
            nc.clear_and_free_semaphores(sems)


def _build(C):
    """Per-core program: y[C, OUT] = xT.T @ w + bias.

    xT   : [P, KO+1, C]          xT[p, ko, c] = x_subject[c, ko*P + p]
                                 for ko < KO; last slot all-ones (bias).
    w    : [N_CHUNKS, P, CH*OUT] host-permuted weights; w[ch, p, j*OUT+n]
                                 = W[(ch*CH + j)*P + p, n].
    bias : [1, OUT]              the subject's bias row.
    """
    cdt = mybir.dt.float32r if USE_FP32R else mybir.dt.float32
    nc = bass.Bass(enable_partition_id=False)
    xT = nc.dram_tensor("xT", [P, KO + 1, C], cdt, kind="ExternalInput")
    w = nc.dram_tensor("w", [N_CHUNKS, P, CH * OUT], cdt, kind="ExternalInput")
    bias = nc.dram_tensor("bias", [1, OUT], cdt, kind="ExternalInput")
    y = nc.dram_tensor("y", [C, OUT], mybir.dt.float32, kind="ExternalOutput")

    m_tiles = [(m0, min(P, C - m0)) for m0 in range(0, C, P)]

    with _FastExitTileContext(nc) as tc:
        with (
            tc.tile_pool(name="wpool", bufs=N_CHUNKS) as wpool,
            tc.tile_pool(name="xpool", bufs=1) as xpool,
            tc.tile_pool(name="bpool", bufs=1) as bpool,
            tc.tile_pool(name="spool", bufs=1) as spool,
            tc.tile_pool(name="opool", bufs=4) as opool,
            tc.tile_pool(name="psum", bufs=1, space="PSUM") as psum_pool,
        ):
            # PE warm-up scratch: memset by GpSimd so the first spin
            # matmul's only wait is the GpSimd semaphore.
            scratch = spool.tile([P, NT], cdt)
            nc.gpsimd.memset(scratch[:], 0.0)

            # Exactly 8 DMAs (x, bias, 4 w chunks, 2 y writes for
            # C <= 128) — one per HWDGE completion-sem lane, so no
            # instruction ever needs a lane-reuse wait.
            x_tile = xpool.tile([P, KO + 1, C], cdt)
            nc.sync.dma_start(x_tile[:], xT[:])
            b_tile = bpool.tile([1, OUT], cdt)
            nc.sync.dma_start(b_tile[:], bias[:])

            # Ring balance: ACT carries x + bias + odd chunks
            # (4.65 MB), SP carries even chunks (4.0 MB) so chunk0 starts
            # transferring immediately and both rings drain until the
            # end. Completion-sem lanes round-robin over 8 in issue
            # order: x=0, bias=1, w0..w5=2..7, w6=0, w7=1. w6/w7 share
            # lanes with x/bias, so their first matmuls wait
            # "lane >= 32" — still a single wait each.
            w_tiles = []
            for ch in range(N_CHUNKS):
                wt = wpool.tile([P, CH * OUT], cdt)
                eng = nc.scalar if ch % 2 == 0 else nc.sync
                eng.dma_start(wt[:], w[ch])
                w_tiles.append(wt)

            # For mc <= 64 the two n-tiles share one PSUM bank on
            # disjoint column halves of the PE array (tile_position), so
            # their matmul streams run concurrently on independent
            # 32x32 sub-arrays.
            col_tiled = all(mc <= 64 for _, mc in m_tiles)
            psums = {}
            tilepos = {}
            for mi, (m0, mc) in enumerate(m_tiles):
                if col_tiled:
                    joint = psum_pool.tile(
                        [P, NT], mybir.dt.float32, name=f"psum_{mi}"
                    )
                    psums[(mi, 0)] = joint[0:mc]
                    psums[(mi, 1)] = joint[64 : 64 + mc]
                    tilepos[(mi, 0)] = (0, 0)
                    tilepos[(mi, 1)] = (0, 64)
                else:
                    for n in range(2):
                        psums[(mi, n)] = psum_pool.tile(
                            [mc, NT], mybir.dt.float32, name=f"psum_{mi}_{n}"
                        )
                        tilepos[(mi, n)] = None
            spin_ps = psum_pool.tile([1, SPIN_N], mybir.dt.float32, name="spin_ps")

            def spin(k):
                for _ in range(k):
                    nc.tensor.matmul(
                        spin_ps[:, :],
                        scratch[:, 0:1],
                        scratch[:, :SPIN_N],
                        start=True,
                        stop=True,
                    )

            spin(SPINS_PRE)
            # Absorbs the x-DMA wait (scratch has no DMA dependency), so
            # later matmuls each need only their own chunk/bias wait.
            warm = psum_pool.tile([1, C], mybir.dt.float32, name="warm")
            nc.tensor.matmul(
                warm[:, :],
                scratch[:, 0:1],
                x_tile[:, 0, :],
                start=True,
                stop=True,
            )
            # Open each accumulation group with the rank-1 bias update:
            # ones[1, mc].T @ bias[1, NT].
            for mi, (m0, mc) in enumerate(m_tiles):
                for n in range(2):
                    nc.tensor.matmul(
                        psums[(mi, n)][:, :],
                        x_tile[0:1, KO, m0 : m0 + mc],
                        b_tile[0:1, n * NT : (n + 1) * NT],
                        start=True,
                        stop=False,
                        tile_position=tilepos[(mi, n)],
                    )
            # k-contiguous loop: each W chunk is consumed for every
            # (m, n) output tile as soon as it lands, then is dead.
            for ko in range(KO):
                wt = w_tiles[ko // CH]
                base = (ko % CH) * OUT
                for mi, (m0, mc) in enumerate(m_tiles):
                    lhsT = x_tile[:, ko, m0 : m0 + mc]
                    for n in range(2):
                        nc.tensor.matmul(
                            psums[(mi, n)][:, :],
                            lhsT,
                            wt[:, base + n * NT : base + (n + 1) * NT],
                            start=False,
                            stop=(ko == KO - 1),
                            tile_position=tilepos[(mi, n)],
                        )
                if SPINS_GAP and ko % CH == CH - 1 and ko != KO - 1:
                    spin(SPINS_GAP)
            for mi, (m0, mc) in enumerate(m_tiles):
                for n in range(2):
                    ot = opool.tile([mc, NT], mybir.dt.float32)
                    nc.vector.tensor_copy(ot[:], psums[(mi, n)][:])
                    # SWDGE: keeps the y writes off the 8 HWDGE
                    # completion-sem lanes (no lane-reuse wait).
                    nc.gpsimd.dma_start(y[m0 : m0 + mc, n * NT : (n + 1) * NT], ot[:])
    return nc


def _capacity(max_count):
    c = 48
    while c < max_count:
        c *= 2
    return c


def kernel(x, subject_ids, W, b):
    global LAST_RESULTS
    x = np.ascontiguousarray(np.asarray(x, dtype=np.float32))
    sid = np.asarray(subject_ids).astype(np.int64)
    W = np.ascontiguousarray(np.asarray(W, dtype=np.float32))
    b = np.ascontiguousarray(np.asarray(b, dtype=np.float32))

    groups = [np.nonzero(sid == s)[0] for s in range(S)]
    C = _capacity(max((len(g) for g in groups), default=1))

    key = (C, USE_FP32R, SPINS_PRE, SPINS_GAP, CH)
    if key not in _nc_cache:
        _nc_cache[key] = _build(C)
    nc = _nc_cache[key]

    # [ch, p, j*OUT + n] = W[s, (ch*CH + j)*P + p, n]: one contiguous
    # 16 KB run per partition per chunk DMA.
    W_perm = np.ascontiguousarray(
        W.reshape(S, N_CHUNKS, CH, P, OUT).transpose(0, 1, 3, 2, 4)
    ).reshape(S, N_CHUNKS, P, CH * OUT)

    in_maps = []
    for s in range(S):
        idx = groups[s]
        xs = np.zeros((C, D), dtype=np.float32)
        xs[: len(idx)] = x[idx]
        # [p, ko, c] = xs[c, ko*P + p]; extra all-ones k-slot for bias
        xT = np.empty((P, KO + 1, C), dtype=np.float32)
        xT[:, :KO, :] = xs.T.reshape(KO, P, C).transpose(1, 0, 2)
        xT[:, KO, :] = 1.0
        in_maps.append({"xT": xT, "w": W_perm[s], "bias": b[s : s + 1]})

    LAST_RESULTS = run_bass_kernel_spmd(
        nc, in_maps, core_ids=list(range(S)), trace=TRACE
    )

    out = np.zeros((B, OUT), dtype=np.float32)
    for s in range(S):
        idx = groups[s]
        out[idx] = LAST_RESULTS.results[s]["y"][: len(idx)]
    return out



# revision 10
# speedup vs baseline: 1.3705x; 1.3705x over previous
"""Trainium2 kernel for per-subject linear heads (moe_routing).

Computes out[i] = x[i] @ W[subject_ids[i]] + b[subject_ids[i]] for
B=256, D=2048, S=8 subjects, OUT=1000.

Sharding: expert-parallel — core s owns subject s. Each core reads only
its own (2048, 1000) weight slice from HBM, so the total weight traffic
across the chip is W read exactly once (vs 8x for batch-data-parallel
with a replicated table). Samples are grouped by subject on the host,
padded to a fixed capacity C, and fed to an SPMD Bass/Tile kernel;
outputs are scattered back to the original order.

Precision: W/x/bias are cast to bf16 on the host. This halves the HBM
stream (4.1 MB of weights per core instead of 8.2 MB) and makes each
matmul single-pass on the PE (fp32 runs in LOW_HIGH two-pass mode, 2x
the cycles). Accumulation stays fp32 in PSUM; measured end-to-end rel
err ~2.4e-3.

Kernel-side notes:
- The bias is folded into the matmul accumulation as a rank-1 update
  (ones row carried as an extra k-slot of x, times the [1, OUT] bias).
- This walrus build rejects any instruction with more than one sync
  wait, so the kernel is structured so no instruction ever needs two:
  a scratch-fed warm-up matmul absorbs the x-DMA wait, the 8 HWDGE
  DMAs (x, bias, 4 W chunks, 2 y writes) map 1:1 onto the 8 HWDGE
  completion-sem lanes, and the custom TileContext tail emits one
  drain per semaphore.
- W is pre-permuted on the host so each chunk DMA reads one contiguous
  8 KB run per partition (128 fat descriptors).
- A chain of throwaway matmuls on a memset scratch tile keeps the PE
  busy from kernel start so the HAM clock-gate reaches 2.4 GHz before
  the real matmul stream begins.
- y writes go out over the two HWDGE rings (0.6 us first-byte vs 1 us
  for SWDGE), one [C, 500] half per ring, right after each PSUM half
  is copied to SBUF.
"""

import ml_dtypes
import numpy as np

import concourse.bass as bass
import concourse.mybir as mybir
import concourse.tile as tile
from concourse.bass_utils import run_bass_kernel_spmd
from concourse.vector_clock import ScopedClock, VectorClock

B = 256
D = 2048
S = 8
OUT = 1000
P = 128
KO = D // P          # 16 k-tiles of 128
NT = 500             # psum n-tile (<= 512 fp32 / bank), 2 tiles cover OUT
CH = 4               # k-tiles per W DMA chunk (4 * 128 * 1000 * 2B = 1 MB)
N_CHUNKS = KO // CH

SPINS_PRE = 16       # PE warm-up matmuls before the real stream
SPIN_N = 128         # spin matmul free dim (short, so cut-over to real work is fast)

TRACE = False        # set by test harness to collect an NTFF profile
LAST_RESULTS = None  # BassKernelResults of the most recent run

_nc_cache = {}


class _FastExitTileContext(tile.TileContext):
    """TileContext with a single-wait-per-instruction, barrier-free exit.

    This walrus build rejects instructions with >1 sync wait, and the
    stock exit (one Drain waiting on every semaphore + two all-engine
    EVSEM-butterfly barriers) both violates that and costs ~8 us. Here
    SP emits one drain per logical processor (each <=1 wait), then
    hands off to GpSimd via a fresh semaphore; GpSimd resets the DMA
    queues and clears all Tile semaphores (required so a re-execution
    of the NEFF starts from zeroed sems). By the time SP's drains have
    observed every semaphore at its final value, every engine has
    retired its last instruction, so the butterfly barriers are
    unnecessary.
    """

    def _drain_and_barrier(self, tick_clock, wait_clock):
        nc = self.nc
        gc = tick_clock.global_clock
        n = len(gc)
        last = None
        for i in range(n):
            if gc[i] <= 0:
                continue
            vec = [0] * n
            vec[i] = gc[i]
            d = nc.sync.drain()
            wait_clock.add_sem_waits(d.ins, ScopedClock({None: VectorClock(vec)}))
            last = d

        assert self.sems is not None
        popped = nc._tile_sem_poison_stack.pop()
        assert popped is self._sem_poison
        sems = list(self.sems.allocated().values())
        if last is not None:
            handoff = nc.alloc_semaphore(name="exit_handoff")
            last.then_inc(handoff, 1)
            nc.gpsimd.wait_ge(handoff, 1)
            nc.clear_and_free_semaphores(sems)
            nc.gpsimd.sem_clear(handoff)
            nc.release_semaphore(handoff)
        else:
            nc.clear_and_free_semaphores(sems)


def _build(C):
    """Per-core program: y[C, OUT] = xT.T @ w + bias.

    xT   : [P, KO+1, C]          xT[p, ko, c] = x_subject[c, ko*P + p]
                                 for ko < KO; last slot all-ones (bias).
    w    : [N_CHUNKS, P, CH*OUT] host-permuted weights; w[ch, p, j*OUT+n]
                                 = W[(ch*CH + j)*P + p, n].
    bias : [1, OUT]              the subject's bias row.
    """
    cdt = mybir.dt.bfloat16
    nc = bass.Bass(enable_partition_id=False)
    xT = nc.dram_tensor("xT", [P, KO + 1, C], cdt, kind="ExternalInput")
    w = nc.dram_tensor("w", [N_CHUNKS, P, CH * OUT], cdt, kind="ExternalInput")
    bias = nc.dram_tensor("bias", [1, OUT], cdt, kind="ExternalInput")
    y = nc.dram_tensor("y", [C, OUT], mybir.dt.float32, kind="ExternalOutput")

    m_tiles = [(m0, min(P, C - m0)) for m0 in range(0, C, P)]

    with _FastExitTileContext(nc) as tc:
        with (
            tc.tile_pool(name="wpool", bufs=N_CHUNKS) as wpool,
            tc.tile_pool(name="xpool", bufs=1) as xpool,
            tc.tile_pool(name="bpool", bufs=1) as bpool,
            tc.tile_pool(name="spool", bufs=1) as spool,
            tc.tile_pool(name="opool", bufs=4) as opool,
            tc.tile_pool(name="psum", bufs=1, space="PSUM") as psum_pool,
        ):
            # PE warm-up scratch: memset by GpSimd so the first spin
            # matmul's only wait is the GpSimd semaphore.
            scratch = spool.tile([P, NT], cdt)
            nc.gpsimd.memset(scratch[:], 0.0)

            x_tile = xpool.tile([P, KO + 1, C], cdt)
            nc.sync.dma_start(x_tile[:], xT[:])
            b_tile = bpool.tile([1, OUT], cdt)
            nc.sync.dma_start(b_tile[:], bias[:])

            # Ring balance: SP carries x + bias + odd chunks (2.25 MB),
            # ACT carries even chunks (2.0 MB) so chunk0 starts
            # transferring immediately and both rings drain until the
            # end. Completion-sem lanes round-robin over 8 in issue
            # order: x=0, bias=1, w0..w3=2..5, y0=6, y1=7 — exactly 8,
            # no lane reuse, so every DMA-dependent wait is single.
            w_tiles = []
            for ch in range(N_CHUNKS):
                wt = wpool.tile([P, CH * OUT], cdt)
                eng = nc.scalar if ch % 2 == 0 else nc.sync
                eng.dma_start(wt[:], w[ch])
                w_tiles.append(wt)

            # For mc <= 64 the two n-tiles share one PSUM bank on
            # disjoint column halves of the PE array (tile_position), so
            # their matmul streams run concurrently on independent
            # 32x32 sub-arrays.
            col_tiled = all(mc <= 64 for _, mc in m_tiles)
            psums = {}
            tilepos = {}
            joints = {}
            for mi, (m0, mc) in enumerate(m_tiles):
                if col_tiled:
                    joint = psum_pool.tile(
                        [P, NT], mybir.dt.float32, name=f"psum_{mi}"
                    )
                    joints[mi] = joint
                    psums[(mi, 0)] = joint[0:mc]
                    psums[(mi, 1)] = joint[64 : 64 + mc]
                    tilepos[(mi, 0)] = (0, 0)
                    tilepos[(mi, 1)] = (0, 64)
                else:
                    for n in range(2):
                        psums[(mi, n)] = psum_pool.tile(
                            [mc, NT], mybir.dt.float32, name=f"psum_{mi}_{n}"
                        )
                        tilepos[(mi, n)] = None
            spin_ps = psum_pool.tile([1, SPIN_N], mybir.dt.float32, name="spin_ps")

            def spin(k):
                for _ in range(k):
                    nc.tensor.matmul(
                        spin_ps[:, :],
                        scratch[:, 0:1],
                        scratch[:, :SPIN_N],
                        start=True,
                        stop=True,
                    )

            spin(SPINS_PRE)
            # Absorbs the x-DMA wait (scratch has no DMA dependency), so
            # later matmuls each need only their own chunk/bias wait.
            warm = psum_pool.tile([1, C], mybir.dt.float32, name="warm")
            nc.tensor.matmul(
                warm[:, :],
                scratch[:, 0:1],
                x_tile[:, 0, :],
                start=True,
                stop=True,
            )
            # Open each accumulation group with the rank-1 bias update:
            # ones[1, mc].T @ bias[1, NT].
            for mi, (m0, mc) in enumerate(m_tiles):
                for n in range(2):
                    nc.tensor.matmul(
                        psums[(mi, n)][:, :],
                        x_tile[0:1, KO, m0 : m0 + mc],
                        b_tile[0:1, n * NT : (n + 1) * NT],
                        start=True,
                        stop=False,
                        tile_position=tilepos[(mi, n)],
                    )
            # k-contiguous loop: each W chunk is consumed for every
            # (m, n) output tile as soon as it lands, then is dead.
            for ko in range(KO):
                wt = w_tiles[ko // CH]
                base = (ko % CH) * OUT
                for mi, (m0, mc) in enumerate(m_tiles):
                    lhsT = x_tile[:, ko, m0 : m0 + mc]
                    for n in range(2):
                        nc.tensor.matmul(
                            psums[(mi, n)][:, :],
                            lhsT,
                            wt[:, base + n * NT : base + (n + 1) * NT],
                            start=False,
                            stop=(ko == KO - 1),
                            tile_position=tilepos[(mi, n)],
                        )
            # One DVE copy drains the whole joint PSUM bank (DVE cost
            # scales with free size, not partitions), then the two y
            # halves go out over both HWDGE rings on fresh completion
            # lanes (6 and 7) — every instruction single-wait.
            for mi, (m0, mc) in enumerate(m_tiles):
                if col_tiled:
                    ot = opool.tile([P, NT], mybir.dt.float32)
                    nc.vector.tensor_copy(ot[:], joints[mi][:])
                    nc.sync.dma_start(y[m0 : m0 + mc, 0:NT], ot[0:mc])
                    nc.scalar.dma_start(
                        y[m0 : m0 + mc, NT : 2 * NT], ot[64 : 64 + mc]
                    )
                else:
                    for n in range(2):
                        ot = opool.tile([mc, NT], mybir.dt.float32)
                        nc.vector.tensor_copy(ot[:], psums[(mi, n)][:])
                        eng = nc.sync if n == 0 else nc.scalar
                        eng.dma_start(
                            y[m0 : m0 + mc, n * NT : (n + 1) * NT], ot[:]
                        )
    return nc


def _capacity(max_count):
    c = 48
    while c < max_count:
        c *= 2
    return c


def kernel(x, subject_ids, W, b):
    global LAST_RESULTS
    x = np.asarray(x, dtype=np.float32)
    sid = np.asarray(subject_ids).astype(np.int64)
    W = np.asarray(W, dtype=np.float32)
    b = np.asarray(b, dtype=np.float32)

    groups = [np.nonzero(sid == s)[0] for s in range(S)]
    C = _capacity(max((len(g) for g in groups), default=1))

    key = (C, CH)
    if key not in _nc_cache:
        _nc_cache[key] = _build(C)
    nc = _nc_cache[key]

    bf16 = ml_dtypes.bfloat16
    # [ch, p, j*OUT + n] = W[s, (ch*CH + j)*P + p, n]: one contiguous
    # 8 KB run per partition per chunk DMA.
    W_perm = np.ascontiguousarray(
        W.astype(bf16).reshape(S, N_CHUNKS, CH, P, OUT).transpose(0, 1, 3, 2, 4)
    ).reshape(S, N_CHUNKS, P, CH * OUT)
    b16 = b.astype(bf16)

    in_maps = []
    for s in range(S):
        idx = groups[s]
        xs = np.zeros((C, D), dtype=np.float32)
        xs[: len(idx)] = x[idx]
        # [p, ko, c] = xs[c, ko*P + p]; extra all-ones k-slot for bias
        xT = np.empty((P, KO + 1, C), dtype=bf16)
        xT[:, :KO, :] = xs.T.reshape(KO, P, C).transpose(1, 0, 2).astype(bf16)
        xT[:, KO, :] = 1.0
        in_maps.append({"xT": xT, "w": W_perm[s], "bias": b16[s : s + 1]})

    LAST_RESULTS = run_bass_kernel_spmd(
        nc, in_maps, core_ids=list(range(S)), trace=TRACE
    )

    out = np.zeros((B, OUT), dtype=np.float32)
    for s in range(S):
        idx = groups[s]
        out[idx] = LAST_RESULTS.results[s]["y"][: len(idx)]
    return out


# revision 11
# speedup vs baseline: 1.4081x; 1.0275x over previous
"""Trainium2 kernel for per-subject linear heads (moe_routing).

Computes out[i] = x[i] @ W[subject_ids[i]] + b[subject_ids[i]] for
B=256, D=2048, S=8 subjects, OUT=1000.

Sharding: expert-parallel — core s owns subject s. Each core reads only
its own (2048, 1000) weight slice from HBM, so the total weight traffic
across the chip is W read exactly once (vs 8x for batch-data-parallel
with a replicated table). Samples are grouped by subject on the host,
padded to a fixed capacity C, and fed to an SPMD Bass/Tile kernel;
outputs are scattered back to the original order.

Precision: W/x/bias are cast to bf16 on the host. This halves the HBM
stream (4.1 MB of weights per core instead of 8.2 MB) and makes each
matmul single-pass on the PE (fp32 runs in LOW_HIGH two-pass mode, 2x
the cycles). Accumulation stays fp32 in PSUM; measured end-to-end rel
err ~2.4e-3 (the harness gate is 2e-2).

Scheduling notes (from trace analysis):
- The two HWDGE rings (SP + ACT) sustain ~420 GB/s aggregate, but a
  DMA's completion semaphore lags its data: each of the 16 SDMA
  engines increments the sem with its LAST descriptor, so a tiny DMA
  queued behind a 1 MB chunk on the same ring only "completes" when
  the chunk's packets drain. Hence x and bias go over SWDGE (gpsimd)
  — a separate engine-internal queue — and the HWDGE rings carry ONLY
  the W chunks, in ring-alternating order.
- W is split into 6 uneven chunks (k-tiles [2,3,3,3,3,2]): a small
  first chunk so the PE starts consuming early, small last chunk so
  the final completion sem fires close to the end of the stream.
- The bias is folded into the matmul accumulation as a rank-1 update
  (ones row carried as an extra k-slot of x, times the [1, OUT] bias).
- This walrus build rejects any instruction with more than one sync
  wait: chunk lanes 0-5 and y lanes 6-7 map 1:1 onto the 8 HWDGE
  completion-sem lanes, x/bias use SWDGE lanes, a scratch-fed warm-up
  matmul absorbs the x wait, and the custom TileContext tail emits
  one drain per semaphore.
- W is pre-permuted on the host so each chunk DMA reads one contiguous
  run per partition (128 fat descriptors per chunk).
- A chain of throwaway matmuls on a memset scratch tile keeps the PE
  busy from kernel start so the HAM clock-gate reaches 2.4 GHz before
  the real matmul stream begins (cold matmuls take 622 ns vs 208 ns).
"""

import ml_dtypes
import numpy as np

import concourse.bass as bass
import concourse.mybir as mybir
import concourse.tile as tile
from concourse.bass_utils import run_bass_kernel_spmd
from concourse.vector_clock import ScopedClock, VectorClock

B = 256
D = 2048
S = 8
OUT = 1000
P = 128
KO = D // P          # 16 k-tiles of 128
NT = 500             # psum n-tile (<= 512 fp32 / bank), 2 tiles cover OUT
CHUNK_KT = (2, 3, 3, 3, 3, 2)   # k-tiles per W chunk; 6 chunks + 2 y = 8 lanes
assert sum(CHUNK_KT) == KO

SPINS_PRE = 28       # PE warm-up matmuls before the real stream
SPIN_N = 128         # spin matmul free dim (short, so cut-over to real work is fast)

TRACE = False        # set by test harness to collect an NTFF profile
LAST_RESULTS = None  # BassKernelResults of the most recent run

_nc_cache = {}


class _FastExitTileContext(tile.TileContext):
    """TileContext with a single-wait-per-instruction, barrier-free exit.

    This walrus build rejects instructions with >1 sync wait, and the
    stock exit (one Drain waiting on every semaphore + two all-engine
    EVSEM-butterfly barriers) both violates that and costs ~8 us. Here
    SP emits one drain per logical processor (each <=1 wait), then
    hands off to GpSimd via a fresh semaphore; GpSimd resets the DMA
    queues and clears all Tile semaphores (required so a re-execution
    of the NEFF starts from zeroed sems). By the time SP's drains have
    observed every semaphore at its final value, every engine has
    retired its last instruction, so the butterfly barriers are
    unnecessary.
    """

    def _drain_and_barrier(self, tick_clock, wait_clock):
        nc = self.nc
        gc = tick_clock.global_clock
        n = len(gc)
        last = None
        for i in range(n):
            if gc[i] <= 0:
                continue
            vec = [0] * n
            vec[i] = gc[i]
            d = nc.sync.drain()
            wait_clock.add_sem_waits(d.ins, ScopedClock({None: VectorClock(vec)}))
            last = d

        assert self.sems is not None
        popped = nc._tile_sem_poison_stack.pop()
        assert popped is self._sem_poison
        sems = list(self.sems.allocated().values())
        if last is not None:
            handoff = nc.alloc_semaphore(name="exit_handoff")
            last.then_inc(handoff, 1)
            nc.gpsimd.wait_ge(handoff, 1)
            nc.clear_and_free_semaphores(sems)
            nc.gpsimd.sem_clear(handoff)
            nc.release_semaphore(handoff)
        else:
            nc.clear_and_free_semaphores(sems)


def _build(C):
    """Per-core program: y[C, OUT] = xT.T @ w + bias.

    xT   : [P, KO+1, C]   xT[p, ko, c] = x_subject[c, ko*P + p]
                          for ko < KO; last slot all-ones (bias).
    w    : [P, KO*OUT]    host-permuted weights; w[p, k*OUT + n]
                          = W[k*P + p, n]. Chunk ch covers k-tiles
                          [k0, k0+kt) as one contiguous per-partition
                          byte range.
    bias : [1, OUT]       the subject's bias row.
    """
    cdt = mybir.dt.bfloat16
    nc = bass.Bass(enable_partition_id=False)
    xT = nc.dram_tensor("xT", [P, KO + 1, C], cdt, kind="ExternalInput")
    w = nc.dram_tensor("w", [P, KO * OUT], cdt, kind="ExternalInput")
    bias = nc.dram_tensor("bias", [1, OUT], cdt, kind="ExternalInput")
    y = nc.dram_tensor("y", [C, OUT], mybir.dt.float32, kind="ExternalOutput")

    m_tiles = [(m0, min(P, C - m0)) for m0 in range(0, C, P)]
    starts = [sum(CHUNK_KT[:i]) for i in range(len(CHUNK_KT))]

    with _FastExitTileContext(nc) as tc:
        with (
            tc.tile_pool(name="wpool", bufs=len(CHUNK_KT)) as wpool,
            tc.tile_pool(name="xpool", bufs=1) as xpool,
            tc.tile_pool(name="bpool", bufs=1) as bpool,
            tc.tile_pool(name="spool", bufs=1) as spool,
            tc.tile_pool(name="opool", bufs=4) as opool,
            tc.tile_pool(name="psum", bufs=1, space="PSUM") as psum_pool,
        ):
            # PE warm-up scratch: memset by GpSimd so the first spin
            # matmul's only wait is the GpSimd semaphore.
            scratch = spool.tile([P, NT], cdt)
            nc.gpsimd.memset(scratch[:], 0.0)

            # W chunks alternate HWDGE rings (SP even, ACT odd); the
            # rings carry nothing else until the y writes, so chunk
            # completion sems fire as soon as each chunk's own packets
            # drain.
            w_tiles = []
            for ch, kt in enumerate(CHUNK_KT):
                wt = wpool.tile([P, kt * OUT], cdt)
                eng = nc.sync if ch % 2 == 0 else nc.scalar
                eng.dma_start(wt[:], w[:, starts[ch] * OUT : (starts[ch] + kt) * OUT])
                w_tiles.append(wt)

            # x + bias ride SWDGE (gpsimd) so their completion is not
            # queued behind megabytes of W packets on the HWDGE rings.
            x_tile = xpool.tile([P, KO + 1, C], cdt)
            nc.gpsimd.dma_start(x_tile[:], xT[:])
            b_tile = bpool.tile([1, OUT], cdt)
            nc.gpsimd.dma_start(b_tile[:], bias[:])

            # For mc <= 64 the two n-tiles share one PSUM bank on
            # disjoint column halves of the PE array (tile_position), so
            # their matmul streams run concurrently on independent
            # 32x32 sub-arrays.
            col_tiled = all(mc <= 64 for _, mc in m_tiles)
            psums = {}
            tilepos = {}
            joints = {}
            for mi, (m0, mc) in enumerate(m_tiles):
                if col_tiled:
                    joint = psum_pool.tile(
                        [P, NT], mybir.dt.float32, name=f"psum_{mi}"
                    )
                    joints[mi] = joint
                    psums[(mi, 0)] = joint[0:mc]
                    psums[(mi, 1)] = joint[64 : 64 + mc]
                    tilepos[(mi, 0)] = (0, 0)
                    tilepos[(mi, 1)] = (0, 64)
                else:
                    for n in range(2):
                        psums[(mi, n)] = psum_pool.tile(
                            [mc, NT], mybir.dt.float32, name=f"psum_{mi}_{n}"
                        )
                        tilepos[(mi, n)] = None
            spin_ps = psum_pool.tile([1, SPIN_N], mybir.dt.float32, name="spin_ps")

            def spin(k):
                for _ in range(k):
                    nc.tensor.matmul(
                        spin_ps[:, :],
                        scratch[:, 0:1],
                        scratch[:, :SPIN_N],
                        start=True,
                        stop=True,
                    )

            spin(SPINS_PRE)
            # Absorbs the x-DMA wait (scratch has no DMA dependency), so
            # later matmuls each need only their own chunk/bias wait.
            warm = psum_pool.tile([1, C], mybir.dt.float32, name="warm")
            nc.tensor.matmul(
                warm[:, :],
                scratch[:, 0:1],
                x_tile[:, 0, :],
                start=True,
                stop=True,
            )
            # Open each accumulation group with the rank-1 bias update:
            # ones[1, mc].T @ bias[1, NT].
            for mi, (m0, mc) in enumerate(m_tiles):
                for n in range(2):
                    nc.tensor.matmul(
                        psums[(mi, n)][:, :],
                        x_tile[0:1, KO, m0 : m0 + mc],
                        b_tile[0:1, n * NT : (n + 1) * NT],
                        start=True,
                        stop=False,
                        tile_position=tilepos[(mi, n)],
                    )
            # k-contiguous loop: each W chunk is consumed for every
            # (m, n) output tile as soon as it lands, then is dead.
            for ch, kt in enumerate(CHUNK_KT):
                wt = w_tiles[ch]
                for j in range(kt):
                    ko = starts[ch] + j
                    for mi, (m0, mc) in enumerate(m_tiles):
                        lhsT = x_tile[:, ko, m0 : m0 + mc]
                        for n in range(2):
                            nc.tensor.matmul(
                                psums[(mi, n)][:, :],
                                lhsT,
                                wt[:, j * OUT + n * NT : j * OUT + (n + 1) * NT],
                                start=False,
                                stop=(ko == KO - 1),
                                tile_position=tilepos[(mi, n)],
                            )
            # One DVE copy drains the whole joint PSUM bank (DVE cost
            # scales with free size, not partitions), then the two y
            # halves go out over both HWDGE rings on fresh completion
            # lanes (6 and 7) — every instruction single-wait.
            for mi, (m0, mc) in enumerate(m_tiles):
                if col_tiled:
                    ot = opool.tile([P, NT], mybir.dt.float32)
                    nc.vector.tensor_copy(ot[:], joints[mi][:])
                    nc.sync.dma_start(y[m0 : m0 + mc, 0:NT], ot[0:mc])
                    nc.scalar.dma_start(
                        y[m0 : m0 + mc, NT : 2 * NT], ot[64 : 64 + mc]
                    )
                else:
                    for n in range(2):
                        ot = opool.tile([mc, NT], mybir.dt.float32)
                        nc.vector.tensor_copy(ot[:], psums[(mi, n)][:])
                        eng = nc.sync if n == 0 else nc.scalar
                        eng.dma_start(
                            y[m0 : m0 + mc, n * NT : (n + 1) * NT], ot[:]
                        )
    return nc


def _capacity(max_count):
    c = 48
    while c < max_count:
        c *= 2
    return c


def kernel(x, subject_ids, W, b):
    global LAST_RESULTS
    x = np.asarray(x, dtype=np.float32)
    sid = np.asarray(subject_ids).astype(np.int64)
    W = np.asarray(W, dtype=np.float32)
    b = np.asarray(b, dtype=np.float32)

    groups = [np.nonzero(sid == s)[0] for s in range(S)]
    C = _capacity(max((len(g) for g in groups), default=1))

    key = (C, CHUNK_KT, SPINS_PRE)
    if key not in _nc_cache:
        _nc_cache[key] = _build(C)
    nc = _nc_cache[key]

    bf16 = ml_dtypes.bfloat16
    # [p, k*OUT + n] = W[s, k*P + p, n]: every chunk DMA reads one
    # contiguous per-partition byte range.
    W_perm = np.ascontiguousarray(
        W.astype(bf16).reshape(S, KO, P, OUT).transpose(0, 2, 1, 3)
    ).reshape(S, P, KO * OUT)
    b16 = b.astype(bf16)

    in_maps = []
    for s in range(S):
        idx = groups[s]
        xs = np.zeros((C, D), dtype=np.float32)
        xs[: len(idx)] = x[idx]
        # [p, ko, c] = xs[c, ko*P + p]; extra all-ones k-slot for bias
        xT = np.empty((P, KO + 1, C), dtype=bf16)
        xT[:, :KO, :] = xs.T.reshape(KO, P, C).transpose(1, 0, 2).astype(bf16)
        xT[:, KO, :] = 1.0
        in_maps.append({"xT": xT, "w": W_perm[s], "bias": b16[s : s + 1]})

    LAST_RESULTS = run_bass_kernel_spmd(
        nc, in_maps, core_ids=list(range(S)), trace=TRACE
    )

    out = np.zeros((B, OUT), dtype=np.float32)
    for s in range(S):
        idx = groups[s]
        out[idx] = LAST_RESULTS.results[s]["y"][: len(idx)]
    return out
